# revision 67
# baseline (speedup 1.0000x reference)
"""Trainium2 Bass kernel for nn_Attention_43963285242601.

GQA attention block: q/k/v projections + RoPE + causal attention + o_proj,
tensor-parallel over 8 NeuronCores.

Sharding (core c of 8):
  - q-heads 4c..4c+3 and kv-head c: Wq/Wk/Wv column (head) shards,
    attention fully local per head group.
  - o_proj sharded over Wo ROWS (output features): every core computes
    out[:, 512c:512c+512] and needs the full attention output, distributed
    via AllGather (bf16).  The host concatenates the 8 feature shards.

v7 (from v6 trace analysis; v6 = 1047us):
  - The single per-batch AllGather exposed a 46.5us PE stall (b1 gather
    triggered at 776us, completed 887us) plus two K=4/8 cold windows.
    Now each batch's gather is split into two token-halves (4 collectives
    of 1MB->8MB each) fired as soon as that half's attention is drained:
    tg3 / tg4 / tg6 / early-tail.  All wire time hides under compute.
  - o_proj emission is interleaved with the tail drain of the last
    attention groups, so the PE chews o_proj matmuls while the exp
    chains of the final b1 groups retire, and the last bounce write
    (which gates the final collective) lands as early as possible.
  - pb_t (reciprocal broadcast) ran as a 1.5us fp32 LOW_HIGH 2-pass
    matmul (48us of PE total); the V transposes were fp32 2-pass too.
    All tail matmuls and transposes now run 1-pass bf16.
  - Causal diagonal trimming: for the 4 diagonal key tiles of each
    512-query group only the visible q-range [128j:512) is computed
    (scores, exp, PV, denominator adds); the mask multiply shrinks to
    the single 128x128 diagonal block.
  - x / Wq / Wk / Wv are pre-converted to bf16 on host: halves the
    64MB x stream + 12MB weight DMA, frees SBUF, enables FWL weight
    loads.  PE rate is unchanged (1 col/cycle for both fp32r and bf16).
  - PV accumulator evacuation moved from ACT to DVE so the softmax exp
    chain (ACT FIFO) is never stuck behind copies.
"""

import numpy as np

import concourse.bacc as bacc
import concourse.mybir as mybir
import concourse.tile as tile
from concourse.bass_utils import run_bass_kernel_spmd

F32 = mybir.dt.float32
BF16 = mybir.dt.bfloat16
AF = mybir.ActivationFunctionType

N_CORES = 8
B, L = 2, 2048
N_HEADS, N_KV = 32, 8
HEAD_DIM = 128
D = N_HEADS * HEAD_DIM
THETA = 500000.0

EXP_BIAS = -8.0


def _rope_tables(t_all, l, dh):
    half = dh // 2
    inv = 1.0 / (THETA ** (np.arange(half, dtype=np.float64) * 2.0 / dh))
    pos = np.arange(t_all, dtype=np.float64) % l
    ang = inv[:, None] * pos[None, :]  # [half, T]
    cos = np.cos(ang)
    sin = np.sin(ang)
    return (
        np.concatenate([cos, cos], 0).astype(np.float32),
        np.concatenate([sin, sin], 0).astype(np.float32),
    )


def _build(n_cores=N_CORES, b=B, l=L, nh=N_HEADS, nkv=N_KV):
    import ml_dtypes

    dh = HEAD_DIM
    d = nh * dh
    t_all = b * l
    hpc = nh // n_cores  # q heads per core
    assert nkv == n_cores, "one kv head per core"
    mpc = d // n_cores  # o_proj output features per core
    kt_d = d // dh  # contraction tiles for projections
    tg_n = t_all // 512  # 512-wide token groups
    ksub = 4  # k-tiles per x subslab load
    nsub = kt_d // ksub
    scale = dh ** -0.5
    lh = l // 2  # token half for chunked collectives

    nc = bacc.Bacc(
        "TRN2", target_bir_lowering=False, debug=False, num_devices=n_cores
    )

    xT = nc.dram_tensor("xT", [d, t_all], BF16, kind="ExternalInput").ap()
    wqT = nc.dram_tensor("wqT", [d, hpc * dh], BF16, kind="ExternalInput").ap()
    wkT = nc.dram_tensor("wkT", [d, dh], BF16, kind="ExternalInput").ap()
    wvT = nc.dram_tensor("wvT", [d, dh], BF16, kind="ExternalInput").ap()
    woT = nc.dram_tensor("woT", [d, mpc], BF16, kind="ExternalInput").ap()
    outT = nc.dram_tensor("outT", [mpc, t_all], F32, kind="ExternalOutput").ap()

    # compile-time constants
    cos_np, sin_np = _rope_tables(t_all, l, dh)
    # 128x128 lower-triangle multiplicative mask for the diagonal block:
    # mask128[k, q] = (k <= q)
    k_idx = np.arange(128)[:, None]
    q_idx = np.arange(128)[None, :]
    mask128_np = (k_idx <= q_idx).astype(ml_dtypes.bfloat16)
    cos_c = nc.inline_tensor(cos_np, name="cos_c").ap()
    sin_c = nc.inline_tensor(sin_np, name="sin_c").ap()
    mask128_c = nc.inline_tensor(mask128_np, name="mask128_c").ap()
    ident_c = nc.inline_tensor(
        np.eye(128, dtype=ml_dtypes.bfloat16), name="ident_c"
    ).ap()
    ones_kb_c = nc.inline_tensor(
        np.ones((128, 1), dtype=ml_dtypes.bfloat16), name="ones_kb_c"
    ).ap()
    ones_rf_c = nc.inline_tensor(
        np.ones((1, 128), dtype=ml_dtypes.bfloat16), name="ones_rf_c"
    ).ap()

    with tile.TileContext(nc) as tc:
        with (
            tc.tile_pool(name="constp", bufs=1) as constp,
            tc.tile_pool(name="kvp", bufs=1) as kvp,
            tc.tile_pool(name="qstp", bufs=8) as qstp,
            tc.tile_pool(name="cspool", bufs=2) as cspool,
            tc.tile_pool(name="ropet", bufs=3) as ropet,
            tc.tile_pool(name="vtst", bufs=2) as vtst,
            tc.tile_pool(name="ppool", bufs=3) as ppool,
            tc.tile_pool(name="accp", bufs=2) as accp,
            tc.tile_pool(name="rsb", bufs=2) as rsb,
            tc.tile_pool(name="obf", bufs=2) as obf,
            tc.tile_pool(name="wopool", bufs=1) as wopool,
            tc.tile_pool(name="dramp", bufs=1, space="DRAM") as dramp,
            tc.tile_pool(name="pspp", bufs=1, space="PSUM") as pspp,
            tc.tile_pool(name="pop", bufs=1, space="PSUM") as pop,
        ):
            mask128 = constp.tile([128, 128], BF16, tag="mask128")
            ident = constp.tile([128, 128], BF16, tag="ident")
            ones_kb = constp.tile([128, 1], BF16, tag="ones_kb")
            ones_rf = constp.tile([1, 128], BF16, tag="ones_rf")
            bias_t = constp.tile([128, 1], F32, tag="bias_t")

            def load_consts():
                # deferred so the first weight/x chunks hit the DMA rings
                # first and the first matmul starts ASAP
                nc.gpsimd.dma_start(mask128[:], mask128_c)
                nc.gpsimd.dma_start(ident[:], ident_c)
                nc.gpsimd.dma_start(ones_kb[:], ones_kb_c)
                nc.gpsimd.dma_start(ones_rf[:], ones_rf_c)
                nc.vector.memset(bias_t[:], EXP_BIAS)

            wo_sb = wopool.tile([128, kt_d, mpc], BF16, tag="wo")

            # per-batch K (rotated, [dh, l], bf16) and Vn ([key, dh], bf16)
            K2 = [kvp.tile([128, l], BF16, tag=f"K{i}", name=f"K{i}") for i in range(2)]
            Vn2 = [
                kvp.tile([128, l], BF16, tag=f"Vn{i}", name=f"Vn{i}")
                for i in range(2)
            ]

            # bounce/gather pairs per (batch, token-half): rows = local heads * dh
            bounce = [
                [
                    dramp.tile([hpc * dh, lh], BF16, tag=f"bounce{bb}_{hf}",
                               name=f"bounce{bb}_{hf}")
                    for hf in range(2)
                ]
                for bb in range(b)
            ]
            gathered = [
                [
                    dramp.tile(
                        [n_cores * hpc * dh, lh], BF16,
                        addr_space="Shared" if n_cores > 4 else "Local",
                        tag=f"gath{bb}_{hf}", name=f"gath{bb}_{hf}"
                    )
                    for hf in range(2)
                ]
                for bb in range(b)
            ]

            def fire_gather(bb, hf):
                nc.gpsimd.collective_compute(
                    "AllGather",
                    mybir.AluOpType.bypass,
                    replica_groups=[list(range(n_cores))],
                    ins=[bounce[bb][hf].opt()],
                    outs=[gathered[bb][hf].opt()],
                )

            wq_r = wqT.rearrange("(k p) m -> p k m", p=128)
            wk_r = wkT.rearrange("(k p) m -> p k m", p=128)
            wv_r = wvT.rearrange("(k p) m -> p k m", p=128)
            xT_r = xT.rearrange("(k p) t -> p k t", p=128)

            # ---- attention micro-unit machinery --------------------------
            pending = []  # FIFO of ((batch, group), closure)
            # score-PSUM pool indirection: units allocate at drain time, so
            # the tail region can swap in a double-buffered pool (region 1
            # has no spare PSUM bank; region 2 does)
            pl = {"psp": pspp}

            def drain(k):
                for _ in range(min(k, len(pending))):
                    pending.pop(0)[1]()

            def drain_all():
                while pending:
                    pending.pop(0)[1]()

            def drain_upto(bb, g):
                while pending and pending[0][0] <= (bb, g):
                    pending.pop(0)[1]()

            def make_group_units(bb, h, g, qt):
                """Attention for (batch bb, local head h, 512-query group g).
                qt: rotated q tile [128, 512] bf16.  Appends units to pending.
                Diagonal key tiles (kt-4g = j in [0,4)) only compute the
                visible q-range [128j:512)."""
                nkt = 4 * g + 4
                K = K2[bb]
                Vn = Vn2[bb]
                po = {}
                acc = {}
                state = {}

                def q0_of(kt):
                    j = kt - 4 * g
                    return 128 * j if j > 0 else 0

                def mk_scores(kt):
                    def u():
                        q0 = q0_of(kt)
                        psp = pl["psp"].tile([128, 512], F32, tag="psp", name="psp")
                        state[("psp", kt)] = psp
                        nc.tensor.matmul(
                            psp[:, q0:512],
                            K[:, kt * 128 : (kt + 1) * 128],
                            qt[:, q0:512],
                            start=True,
                            stop=True,
                            skip_group_check=True,
                        )
                    return u

                def mk_softpv(kt):
                    def u():
                        q0 = q0_of(kt)
                        psp = state.pop(("psp", kt))
                        P = ppool.tile([128, 512], BF16, tag="P", name="P")
                        nc.scalar.activation(
                            P[:, q0:512], psp[:, q0:512], AF.Exp,
                            scale=scale, bias=bias_t[:]
                        )
                        j = kt - 4 * g
                        if j >= 0:
                            # mask only the 128-wide diagonal block
                            nc.vector.tensor_mul(
                                P[:, q0 : q0 + 128],
                                P[:, q0 : q0 + 128],
                                mask128[:],
                            )
                        if kt == 0:
                            po["t"] = pop.tile([128, 512], F32, tag="po",
                                               name="po")
                            acc["t"] = accp.tile([128, 512], BF16, tag="acc",
                                                 name="acc")
                        nc.tensor.matmul(
                            po["t"][:, q0:512],
                            Vn[:, kt * 128 : (kt + 1) * 128],
                            P[:, q0:512],
                            start=(kt == 0),
                            stop=(kt == nkt - 1),
                            skip_group_check=True,
                        )
                        if kt == 0:
                            nc.vector.tensor_copy(acc["t"][:], P[:])
                        else:
                            nc.vector.tensor_add(
                                acc["t"][:, q0:512],
                                acc["t"][:, q0:512],
                                P[:, q0:512],
                            )
                        if kt == nkt - 1:
                            # evacuate po (DVE) so its bank frees before the
                            # tail and the ACT FIFO stays clear for exps
                            po["sb"] = obf.tile([128, 512], F32, tag="posb",
                                                name="posb")
                            nc.vector.tensor_copy(po["sb"][:], po["t"][:])
                    return u

                def tail():
                    # denominator sum + fast reciprocal + broadcast; the
                    # two matmuls borrow the score PSUM slot (all bf16,
                    # 1-pass)
                    pd_t = pop.tile([1, 512], F32, tag="po", name="pd")
                    nc.tensor.matmul(
                        pd_t[:], ones_kb[:], acc["t"][:],
                        start=True, stop=True, skip_group_check=True,
                    )
                    r_sb = rsb.tile([1, 512], F32, tag="r", name="r")
                    nc.vector.tensor_copy(r_sb[:], pd_t[:])
                    rr = rsb.tile([1, 512], F32, tag="rr", name="rr")
                    nc.vector.reciprocal_approx_fast(rr[:], r_sb[:])
                    rrb = rsb.tile([1, 512], BF16, tag="rrb", name="rrb")
                    nc.vector.tensor_copy(rrb[:], rr[:])
                    pb_t = pop.tile([128, 512], F32, tag="po", name="pb")
                    nc.tensor.matmul(
                        pb_t[:], ones_rf[:], rrb[:],
                        start=True, stop=True, skip_group_check=True,
                    )
                    ob = obf.tile([128, 512], BF16, tag="ob", name="ob")
                    nc.vector.tensor_mul(ob[:], po["sb"][:], pb_t[:])
                    nc.sync.dma_start(
                        bounce[bb][g // 2][
                            h * dh : (h + 1) * dh,
                            (g % 2) * 512 : (g % 2) * 512 + 512,
                        ],
                        ob[:],
                    )

                # scores run one key-tile ahead of softmax+PV so exp latency
                # is always covered by in-flight PE work
                units = [mk_scores(0)]
                for kt in range(1, nkt):
                    units.append(mk_scores(kt))
                    units.append(mk_softpv(kt - 1))
                units.append(mk_softpv(nkt - 1))
                units.append(tail)
                pending.extend(((bb, g), u) for u in units)

            # ---- fused projection + attention region ---------------------
            with (
                tc.tile_pool(name="wpool", bufs=1) as wpool,
                tc.tile_pool(name="xpool", bufs=4) as xpool,
                tc.tile_pool(name="psq", bufs=1, space="PSUM") as psq,
            ):
                wq_sb = wpool.tile([128, kt_d, hpc * dh], BF16, tag="wq")
                wk_sb = wpool.tile([128, kt_d, dh], BF16, tag="wk")
                wv_sb = wpool.tile([128, kt_d, dh], BF16, tag="wv")

                def emit_tg(tg):
                    bb = tg // 4
                    g = tg % 4
                    toff = tg * 512
                    tloc = g * 512

                    cos_sb = cspool.tile([128, 512], F32, tag="cos")
                    sin_sb = cspool.tile([128, 512], F32, tag="sin")

                    def load_cs():
                        nc.gpsimd.dma_start(cos_sb[:], cos_c[:, toff : toff + 512])
                        nc.gpsimd.dma_start(sin_sb[:], sin_c[:, toff : toff + 512])

                    if tg > 0:
                        load_cs()
                    if tg == 1:
                        # o_proj weights: fire after tg0's weight/x DMAs so
                        # they don't delay the first matmuls
                        nc.gpsimd.dma_start(
                            wo_sb[:], woT.rearrange("(k p) m -> p k m", p=128)
                        )

                    pq = [
                        psq.tile([128, 512], F32, tag=f"pq{o}", name=f"pq{o}")
                        for o in range(hpc)
                    ]
                    pk = psq.tile([128, 512], F32, tag="pk")
                    pv = psq.tile([128, 512], F32, tag="pv")
                    blocks = (
                        [(pk, wk_sb, 0), (pv, wv_sb, 0)]
                        + [(pq[o], wq_sb, o * dh) for o in range(hpc)]
                    )
                    for sub in range(nsub):
                        ks = slice(sub * ksub, (sub + 1) * ksub)
                        xs = xpool.tile([128, ksub, 512], BF16, tag="xs")
                        if tg == 0 and sub == 0:
                            # 1-ktile DMA granularity so the very first
                            # matmul waits on the smallest possible load
                            for k in range(ksub):
                                kk = slice(k, k + 1)
                                nc.gpsimd.dma_start(
                                    wq_sb[:, kk, :], wq_r[:, kk, :]
                                )
                                nc.gpsimd.dma_start(
                                    wk_sb[:, kk, :], wk_r[:, kk, :]
                                )
                                nc.gpsimd.dma_start(
                                    wv_sb[:, kk, :], wv_r[:, kk, :]
                                )
                                nc.sync.dma_start(
                                    xs[:, kk, :], xT_r[:, kk, toff : toff + 512]
                                )
                        else:
                            if tg == 0 and sub == 2:
                                # constants + rope tables: needed only from
                                # the first rope (~end of tg0); keep them
                                # behind the first two weight sub-slabs
                                load_consts()
                                load_cs()
                            if tg == 0:
                                nc.gpsimd.dma_start(wq_sb[:, ks, :], wq_r[:, ks, :])
                                nc.gpsimd.dma_start(wk_sb[:, ks, :], wk_r[:, ks, :])
                                nc.gpsimd.dma_start(wv_sb[:, ks, :], wv_r[:, ks, :])
                            nc.sync.dma_start(xs[:], xT_r[:, ks, toff : toff + 512])
                        for dst, w_sb, o0 in blocks:
                            for k in range(ksub):
                                kt = sub * ksub + k
                                nc.tensor.matmul(
                                    dst[:], w_sb[:, kt, o0 : o0 + dh],
                                    xs[:, k, :],
                                    start=(kt == 0), stop=(kt == kt_d - 1),
                                )
                            drain(3 if g == 3 else 2)

                    def rope(dst_ap, src_ap):
                        # dst[0:64]  = s[0:64]*cos - s[64:]*sin
                        # dst[64:]   = s[64:]*cos + s[0:64]*sin  (out bf16)
                        tc_ = ropet.tile([128, 512], F32, tag="rtc", name="rtc")
                        ts_ = ropet.tile([128, 512], F32, tag="rts", name="rts")
                        nc.vector.tensor_mul(tc_[:], src_ap, cos_sb[:])
                        nc.vector.tensor_mul(
                            ts_[0:64, :], src_ap[64:128, :], sin_sb[64:128, :]
                        )
                        nc.vector.tensor_mul(
                            ts_[64:128, :], src_ap[0:64, :], sin_sb[0:64, :]
                        )
                        nc.vector.tensor_sub(
                            dst_ap[0:64, :], tc_[0:64, :], ts_[0:64, :]
                        )
                        nc.vector.tensor_add(
                            dst_ap[64:128, :], tc_[64:128, :], ts_[64:128, :]
                        )

                    # k: rope from PSUM into K2 (bf16)
                    rope(K2[bb][:, tloc : tloc + 512], pk[:])
                    # v: ACT copy + PE transpose into Vn (transposes borrow
                    # the score PSUM slot)
                    vt = vtst.tile([128, 512], BF16, tag="vt", name="vt")
                    nc.scalar.activation(vt[:], pv[:], AF.Copy)
                    pt = pspp.tile([128, 512], BF16, tag="psp", name="pt")
                    for j in range(4):
                        nc.tensor.transpose(
                            pt[:, j * 128 : (j + 1) * 128],
                            vt[:, j * 128 : (j + 1) * 128],
                            ident[:],
                        )
                    nc.vector.tensor_copy(Vn2[bb][:, tloc : tloc + 512], pt[:])

                    # q ropes (bf16) + enqueue this token group's attention
                    qts = [qstp.tile([128, 512], BF16, tag="qst", name="qst")
                           for _ in range(hpc)]
                    for o in range(hpc):
                        rope(qts[o][:], pq[o][:])
                        make_group_units(bb, o, g, qts[o])
                        drain(2)

                for tg in range(tg_n):
                    emit_tg(tg)
                    # chunked collectives: fire each (batch, token-half)
                    # AllGather as soon as its two groups are drained
                    if tg == 3:
                        drain_upto(0, 1)
                        fire_gather(0, 0)
                    elif tg == 4:
                        drain_upto(0, 3)
                        fire_gather(0, 1)
                    elif tg == 6:
                        drain_upto(1, 1)
                        fire_gather(1, 0)

            # wq/wk/wv/xs pools and proj PSUM released here.
            # ---- tail: interleave remaining attention with o_proj --------
            slabs = [(bb, tgl) for bb in range(b) for tgl in range(l // 512)]
            og_tiles = {}
            state = {"b1_fired": False}

            def maybe_fire_b1():
                if not state["b1_fired"] and not pending:
                    fire_gather(1, 1)
                    state["b1_fired"] = True

            def do_tail(ogpool, outst, pso):
                def og_dma(si):
                    bb, tgl = slabs[si]
                    g_r = gathered[bb][tgl // 2][:].rearrange(
                        "(k p) t -> p k t", p=128
                    )
                    tl = (tgl % 2) * 512
                    og = ogpool.tile([128, kt_d, 512], BF16, tag="og", name="og")
                    # two half-tile triggers: slice-level dependency tracking
                    # lets the first m-block start once kt 0-15 (2MB) have
                    # landed instead of waiting for the full 4MB
                    hk = kt_d // 2
                    nc.gpsimd.dma_start(
                        og[:, 0:hk, :], g_r[:, 0:hk, tl : tl + 512]
                    )
                    nc.gpsimd.dma_start(
                        og[:, hk:kt_d, :], g_r[:, hk:kt_d, tl : tl + 512]
                    )
                    og_tiles[si] = og

                og_dma(0)
                # head drain: covers the og(0) first-half DMA and pushes the
                # last attention groups (which gate the final collective)
                drain(24)
                maybe_fire_b1()
                for si, (bb, tgl) in enumerate(slabs):
                    if si + 1 < len(slabs):
                        og_dma(si + 1)
                    og = og_tiles.pop(si)
                    for m in range(mpc // 128):
                        drain(8)
                        maybe_fire_b1()
                        pp = pso.tile([128, 512], F32, tag="pp", name="pp")
                        for kt in range(kt_d):
                            nc.tensor.matmul(
                                pp[:],
                                wo_sb[:, kt, m * 128 : (m + 1) * 128],
                                og[:, kt, :],
                                start=(kt == 0),
                                stop=(kt == kt_d - 1),
                            )
                        ot = outst.tile([128, 512], F32, tag="ot", name="ot")
                        nc.scalar.activation(ot[:], pp[:], AF.Copy)
                        nc.sync.dma_start(
                            outT[
                                m * 128 : (m + 1) * 128,
                                bb * l + tgl * 512 : bb * l + (tgl + 1) * 512,
                            ],
                            ot[:],
                        )
                    if si == 1 and not state["b1_fired"]:
                        # safety: force the final collective well before
                        # o_proj needs gathered[1][1] (slabs 6-7)
                        drain_all()
                        maybe_fire_b1()

            with (
                tc.tile_pool(name="ogpool", bufs=2) as ogpool,
                tc.tile_pool(name="outst", bufs=3) as outst,
                tc.tile_pool(name="pso", bufs=2, space="PSUM") as pso,
                tc.tile_pool(name="pspp2", bufs=2, space="PSUM") as pspp2,
            ):
                # tail attention scores get a double-buffered PSUM pool so
                # the drained exp chains never serialize the PE queue
                pl["psp"] = pspp2
                do_tail(ogpool, outst, pso)

    nc.compile()
    return nc


_NC_CACHE = {}


def _get_nc(key=(N_CORES, B, L, N_HEADS, N_KV)):
    if key not in _NC_CACHE:
        _NC_CACHE[key] = _build(*key)
    return _NC_CACHE[key]


def make_in_maps(x, Wq, Wk, Wv, Wo, n_cores=N_CORES):
    import ml_dtypes

    b, l, d = x.shape
    nh = Wq.shape[0] // HEAD_DIM
    hpc = nh // n_cores
    mpc = d // n_cores
    xT = np.ascontiguousarray(
        x.reshape(b * l, d).T.astype(ml_dtypes.bfloat16)
    )
    in_maps = []
    for c in range(n_cores):
        wq_c = np.ascontiguousarray(
            Wq[c * hpc * HEAD_DIM : (c + 1) * hpc * HEAD_DIM, :].T.astype(
                ml_dtypes.bfloat16
            )
        )
        wk_c = np.ascontiguousarray(
            Wk[c * HEAD_DIM : (c + 1) * HEAD_DIM, :].T.astype(ml_dtypes.bfloat16)
        )
        wv_c = np.ascontiguousarray(
            Wv[c * HEAD_DIM : (c + 1) * HEAD_DIM, :].T.astype(ml_dtypes.bfloat16)
        )
        wo_c = np.ascontiguousarray(
            Wo[c * mpc : (c + 1) * mpc, :].T.astype(ml_dtypes.bfloat16)
        )
        in_maps.append(
            {"xT": xT, "wqT": wq_c, "wkT": wk_c, "wvT": wv_c, "woT": wo_c}
        )
    return in_maps


def assemble_out(results, b, l, d):
    parts = [r["outT"] for r in results]
    outT = np.concatenate(parts, axis=0)  # [D, T]
    return np.ascontiguousarray(outT.T).reshape(b, l, d).astype(np.float32)


def kernel(x, Wq, Wk, Wv, Wo, trace=False, tmpdir=None):
    x = np.asarray(x, dtype=np.float32)
    nc = _get_nc()
    in_maps = make_in_maps(x, Wq, Wk, Wv, Wo)
    res = run_bass_kernel_spmd(
        nc, in_maps, list(range(N_CORES)), trace=trace, tmpdir=tmpdir
    )
    out = assemble_out(res.results, *x.shape)
    if trace:
        return out, res
    return out


if __name__ == "__main__":
    rng = np.random.default_rng(0)
    s = 0.02
    x = rng.standard_normal((B, L, D)).astype(np.float32)
    Wq = (rng.standard_normal((D, D)) * s).astype(np.float32)
    Wk = (rng.standard_normal((N_KV * HEAD_DIM, D)) * s).astype(np.float32)
    Wv = (rng.standard_normal((N_KV * HEAD_DIM, D)) * s).astype(np.float32)
    Wo = (rng.standard_normal((D, D)) * s).astype(np.float32)
    out = kernel(x, Wq, Wk, Wv, Wo)
    print(out.shape, out.dtype)
